# revision 5
# baseline (speedup 1.0000x reference)
"""NVFP4 block-quantized linear layer (x @ w.T + bias) on 8 Trainium2 cores.

Reference semantics (reference.py): both activations and weights are
block-quantized along K (blocks of 16) to fp4-e2m1 with e4m3 scales
(scale = absmax/6, RNE), dequantized, then matmul with fp32 accumulation,
cast to bf16, plus bf16 bias.

v2 design (per core, 2-way M x 4-way N grid, out stored transposed [n, m]):
  - quantize x rows / w rows on VectorE+GpSimd: blockwise absmax reduce on
    GPSIMD, e4m3 RNE scale via exponent-mask + per-element magic-add
    (bitwise-identical to the reference chain, no reciprocal), fp4 round via
    two custom DVE ops, dequant to bf16 (exactly representable).
  - xdq/wdq staged to DRAM bf16, read back TRANSPOSED via the DMA xbar
    (dma_start_transpose) -- the PE does *only* matmuls (no transposes).
  - stationary = wdqT tile [k,128n], moving = xdqT [k, 512m] chunks; PSUM
    [n=128, m=512] fp32 accumulated over all 24 k-chunks; 8 banks = 2
    nt-pairs in flight (double-buffered).
  - evac: ScalarE activation(Identity) fuses fp32->bf16 cast + per-partition
    bias add (bias is per-n = per-partition in this orientation).
  - out written transposed [N_CORE, M_CORE]; the host reassembles.
"""

import os
import numpy as np
import ml_dtypes

f32 = np.float32
bf16 = ml_dtypes.bfloat16

# ---------------------------------------------------------------------------
# problem geometry (hardcoded; harness calls kernel() with these full shapes)
B, T, K = 2, 4096, 3072
N = 12288
M = B * T                      # 8192
GRID_M, GRID_N = 2, 4          # 8 cores
M_CORE = M // GRID_M           # 4096
N_CORE = N // GRID_N           # 3072
NUM_CORES = GRID_M * GRID_N

KC = K // 128                  # 24 k-chunks
KB = K // 16                   # 192 scale blocks per row
NT = N_CORE // 128             # 24 n-tiles
NPAIR = NT // 2                # 12 nt-pairs
QM = 4                         # m quarters
QSIZE = M_CORE // QM           # 1024
XT_Q = QSIZE // 128            # 8 x row-tiles per quarter

CH1 = float(1.5 * 2**22)
RCP6 = float(f32(1.0) / f32(6.0))
GPSIMD_REDUCE = False          # gpsimd tensor_reduce is partition-axis only

_BUILT = None


# ---------------------------------------------------------------------------
def _register_custom_ops():
    """Register the two fp4-rounding custom DVE ops (idempotent)."""
    import concourse.dve_ops as dve_ops
    from concourse.dve_ops import DveOp, OPS, _SUB_OPCODE_FOR_NAME, _CUSTOM_DVE_ROW_BASE
    from concourse.dve_spec import (
        Spec, Src0, Src1, C0, C1, Zero, One, AluOp, Bin,
        maxx, minn, select, lower, _has_src1,
    )
    from concourse.dve_uop import DveOpSpec

    def _norm2(in0, in1):
        in0 = np.asarray(in0)
        in1 = np.asarray(in1)
        if in1.size != in0.size:
            in1 = np.broadcast_to(in1, in0.shape)
        return in0, np.ascontiguousarray(in1).reshape(in0.shape)

    def _ref_fp4_pre(in0, in1, s0, s1, imm2=None):
        in0, in1 = _norm2(in0, in1)
        m = (in0.astype(f32) * in1.astype(f32)).astype(f32)
        s2 = (m * m).astype(f32)
        ch = np.where(
            s2 < f32(4.0), f32(CH1),
            ((f32(1.0) + (s2 >= f32(16.0)).astype(f32)) * f32(1.5 * 2**23)).astype(f32),
        ).astype(f32)
        return (m + ch).astype(f32)

    def _ref_fp4_fin(in0, in1, s0, s1, imm2=None):
        in0, in1 = _norm2(in0, in1)
        qpre = np.ascontiguousarray(in0.astype(f32))
        pe = (qpre.view(np.uint32) & np.uint32(0x7F800000)).view(f32)
        d1 = (qpre - pe).astype(f32)
        q2 = ((d1 + d1).astype(f32) - pe).astype(f32)
        qc = np.maximum(np.minimum(q2, f32(12.0)), f32(-12.0))
        return (qc * in1.astype(f32)).astype(f32)

    def build_pre():
        SIXTEEN = C0 * C0
        Ch2x = C1 + C1
        m = Src0 * Src1
        s2 = m * m
        c2 = s2 >= SIXTEEN
        inner = (c2 + One) * Ch2x
        c1 = s2 < C0
        outer = select(c1, C1, inner)
        return Spec(body=m + outer, reference=_ref_fp4_pre)

    def build_fin():
        pe = Bin(AluOp.BITWISE_AND, Src0, C0)
        d1 = Src0 - pe
        q2 = (d1 + d1) - pe
        qc = maxx(minn(q2, C1), Zero - C1)
        return Spec(body=qc * Src1, reference=_ref_fp4_fin)

    def register(name, spec):
        if name in _SUB_OPCODE_FOR_NAME:
            for op in OPS:
                if op.name == name:
                    return op
            raise RuntimeError(name)
        row = _CUSTOM_DVE_ROW_BASE + len(OPS)
        assert row < 0x20
        shas = {}
        for ver in ("v3", "v4"):
            try:
                uops = lower(spec, ver=ver)
            except Exception:
                continue
            shas[ver] = DveOpSpec(
                name=name, opcode=row, uops=uops, rd1_en=_has_src1(spec)
            ).sha(ver)
        op = DveOp(name, spec, subdim=False, uops_sha=shas)
        OPS.append(op)
        _SUB_OPCODE_FOR_NAME[name] = row
        dve_ops.CUSTOM_DVE_SPECS[name] = spec
        return op

    return register("FP4_PRE_ANT", build_pre()), register("FP4_FIN_ANT", build_fin())


# ---------------------------------------------------------------------------
def _patch_tile_drain():
    """The TileContext tail drain attaches one sem-wait per live logical
    processor to a single SP Drain instruction; this walrus build caps sync
    waits per instruction at 2 ("Too many sync wait commands").  Split the
    overflow waits onto preceding single-wait SP nops (sound: all waits still
    complete before the post-drain all-engine barrier / sem reset)."""
    from concourse import tile as tile_mod
    import concourse.mybir as mybir
    from concourse.vector_clock import ScopedClock

    if getattr(tile_mod.TileContext, "_ant_drain_patched", False):
        return

    def _drain_and_barrier(self, tick_clock, wait_clock):
        nc = self.nc
        probe = nc.sync.nop()
        wait_clock.add_sem_waits(
            probe.ins, ScopedClock({None: tick_clock.global_clock})
        )
        si = probe.ins.sync_info
        waits = list(si.on_wait) if si is not None and si.on_wait else []
        if len(waits) > 1:
            probe.ins.sync_info = mybir.SyncInfo(
                on_wait=waits[:1],
                on_update=list(si.on_update) if si.on_update else [],
            )
            for w in waits[1:]:
                extra = nc.sync.nop()
                extra.ins.sync_info = mybir.SyncInfo(on_wait=[w], on_update=[])
        nc.sync.drain()

        nc.all_engine_barrier()
        assert self.sems is not None
        popped = nc._tile_sem_poison_stack.pop()
        assert popped is self._sem_poison
        nc.clear_and_free_semaphores(list(self.sems.allocated().values()))
        nc.all_engine_barrier()

    tile_mod.TileContext._drain_and_barrier = _drain_and_barrier
    tile_mod.TileContext._ant_drain_patched = True


def _split_excess_waits(nc, max_waits=1):
    """This walrus build rejects instructions carrying more than `max_waits`
    sem waits ("Too many sync wait commands").  Move overflow waits onto
    same-engine NoOp instructions inserted immediately before the offender —
    per-engine program order makes this semantically identical."""
    import concourse.mybir as mybir

    ctr = [0]
    for f in nc.m.functions:
        for blk in f.blocks:
            il = blk.instructions
            out = []
            changed = False
            for ins in il:
                si = ins.sync_info
                waits = list(si.on_wait) if si is not None and si.on_wait else []
                if len(waits) > max_waits:
                    changed = True
                    extra = waits[:-max_waits]
                    for i0 in range(0, len(extra), max_waits):
                        nop = mybir.InstNoOp(
                            name=f"I-waitsplit-{ctr[0]}", ins=[], outs=[])
                        ctr[0] += 1
                        nop.engine = ins.engine
                        nop.sync_info = mybir.SyncInfo(
                            on_wait=extra[i0:i0 + max_waits], on_update=[])
                        out.append(nop)
                    ins.sync_info = mybir.SyncInfo(
                        on_wait=waits[-max_waits:],
                        on_update=list(si.on_update) if si.on_update else [],
                    )
                out.append(ins)
            if changed:
                blk.instructions = out


def build_nc(m_core=M_CORE, k=K, n_core=N_CORE, num_cores=NUM_CORES,
             debug=False, postprocess=True, gpsimd_reduce=GPSIMD_REDUCE):
    """Build the per-core Bass program (SPMD: same program on every core)."""
    import concourse.bass as bass
    import concourse.mybir as mybir
    from concourse import tile
    from contextlib import ExitStack

    fp4_pre, fp4_fin = _register_custom_ops()
    _patch_tile_drain()

    nc = bass.Bass("TRN2", target_bir_lowering=False, debug=debug,
                   num_devices=num_cores)
    dt = mybir.dt
    Alu = mybir.AluOpType
    Act = mybir.ActivationFunctionType

    x_d = nc.dram_tensor("x", [m_core, k], dt.float32, kind="ExternalInput")
    w_d = nc.dram_tensor("w", [n_core, k], dt.float32, kind="ExternalInput")
    b_d = nc.dram_tensor("bias", [n_core], dt.bfloat16, kind="ExternalInput")
    out_d = nc.dram_tensor("out", [n_core, m_core], dt.bfloat16,
                           kind="ExternalOutput")

    with tile.TileContext(nc) as tc, ExitStack() as ctx:
        dram = ctx.enter_context(tc.tile_pool(name="dram", bufs=1, space="DRAM"))
        qin = ctx.enter_context(tc.tile_pool(name="qin", bufs=2))
        qout = ctx.enter_context(tc.tile_pool(name="qout", bufs=2))
        qpre = ctx.enter_context(tc.tile_pool(name="qpre", bufs=2))
        qtmp = ctx.enter_context(tc.tile_pool(name="qtmp", bufs=2))
        xq = ctx.enter_context(tc.tile_pool(name="xq", bufs=2))
        wT = ctx.enter_context(tc.tile_pool(name="wT", bufs=48))
        osb = ctx.enter_context(tc.tile_pool(name="osb", bufs=3))
        cst = ctx.enter_context(tc.tile_pool(name="cst", bufs=1))
        ps = ctx.enter_context(tc.tile_pool(name="ps", bufs=2, space="PSUM"))

        xdq_d = dram.tile([m_core, k], dt.bfloat16)
        wdq_d = dram.tile([n_core, k], dt.bfloat16)

        # +inf per-partition scalar for FP4_FIN's exponent mask (an inf
        # *immediate* is not JSON-serializable through walrus)
        inf_t = cst.tile([128, 1], dt.float32, tag="inf")
        nc.vector.memset(inf_t[:, :], float("inf"))
        bias_t = cst.tile([128, NT], dt.bfloat16, tag="bias")
        nc.sync.dma_start(
            out=bias_t[:, :], in_=b_d[:].rearrange("(a p) -> p a", p=128))

        def quant(r0, src_d, dst_d):
            """Quantize-dequantize rows [r0, r0+128) of src_d into dst_d."""
            xt = qin.tile([128, k], dt.float32, tag="qin", name="xt")
            nc.scalar.dma_start(out=xt[:, :], in_=src_d[r0:r0 + 128, :])
            x3 = xt[:, :].rearrange("p (b e) -> p b e", e=16)
            bm = qtmp.tile([128, KB], dt.float32, tag="bm", name="bm")
            red_eng = nc.gpsimd if gpsimd_reduce else nc.vector
            red_eng.tensor_reduce(
                bm[:, :], x3, axis=mybir.AxisListType.X, op=Alu.max,
                apply_absolute_value=True,
            )
            # e4m3 RNE scale: sraw = max(bm/6, 2^-9); pe2 = 2^clip(expo,-6..);
            # s = (sraw + 1.5*2^20*pe2) - 1.5*2^20*pe2   (RNE to 2^(e-3) steps)
            sraw = qtmp.tile([128, KB], dt.float32, tag="sraw", name="sraw")
            nc.vector.tensor_scalar(
                sraw[:, :], bm[:, :], RCP6, float(2.0**-9), Alu.mult, Alu.max)
            pe = qtmp.tile([128, KB], dt.float32, tag="pe", name="pe")
            nc.vector.tensor_scalar(
                pe[:, :].bitcast(dt.int32), sraw[:, :].bitcast(dt.int32),
                0x7F800000, None, Alu.bitwise_and)
            pe2 = qtmp.tile([128, KB], dt.float32, tag="pe2", name="pe2")
            nc.vector.tensor_scalar_max(pe2[:, :], pe[:, :], float(2.0**-6))
            mt = qtmp.tile([128, KB], dt.float32, tag="mt", name="mt")
            nc.vector.tensor_scalar_mul(mt[:, :], pe2[:, :], float(1.5 * 2**20))
            yt = qtmp.tile([128, KB], dt.float32, tag="yt", name="yt")
            nc.vector.tensor_tensor(yt[:, :], sraw[:, :], mt[:, :], Alu.add)
            st = qtmp.tile([128, KB], dt.float32, tag="st", name="st")
            nc.vector.tensor_tensor(st[:, :], yt[:, :], mt[:, :], Alu.subtract)
            sh = qtmp.tile([128, KB], dt.float32, tag="sh", name="sh")
            nc.vector.tensor_scalar_mul(sh[:, :], st[:, :], 0.5)
            rinv = qtmp.tile([128, KB], dt.float32, tag="rinv", name="rinv")
            nc.vector.reciprocal(rinv[:, :], st[:, :])

            qp = qpre.tile([128, k], dt.float32, tag="qpre", name="qp")
            qp3 = qp[:, :].rearrange("p (b e) -> p b e", e=16)
            nc.vector._custom_dve(
                fp4_pre, out=qp3, in0=x3,
                in1=rinv[:, :].unsqueeze(2).broadcast_to([128, KB, 16]),
                s0=4.0, s1=CH1,
            )
            dq = qout.tile([128, k], dt.bfloat16, tag="qout", name="dq")
            dq3 = dq[:, :].rearrange("p (b e) -> p b e", e=16)
            nc.vector._custom_dve(
                fp4_fin, out=dq3, in0=qp3,
                in1=sh[:, :].unsqueeze(2).broadcast_to([128, KB, 16]),
                s0=inf_t[:, 0:1], s1=12.0,
            )
            nc.scalar.dma_start(out=dst_d[r0:r0 + 128, :], in_=dq[:, :])

        # ---- wdqT / xqT transposed-read helpers (DMA xbar) ----------------
        pend_wT = {}

        def emit_wT_reads(pair):
            """Prefetch the 24 [128k, 256n] stationary tiles for an nt-pair."""
            tiles = []
            for kc in range(KC):
                wt = wT.tile([128, 256], dt.bfloat16, tag="wT",
                             name=f"wt{pair}_{kc}")
                nc.sync.dma_start_transpose(
                    out=wt[:, :],
                    in_=wdq_d[pair * 256:(pair + 1) * 256,
                              kc * 128:(kc + 1) * 128])
                tiles.append(wt)
            pend_wT[pair] = tiles

        xq_tiles = {}

        def emit_xq_reads(q):
            """Read quarter q of xdq transposed: 24 slices [128k, 1024m]."""
            t = xq.tile([128, KC * QSIZE], dt.bfloat16, tag="xq",
                        name=f"xq{q}")
            for kc in range(KC):
                nc.sync.dma_start_transpose(
                    out=t[:, kc * QSIZE:(kc + 1) * QSIZE],
                    in_=xdq_d[q * QSIZE:(q + 1) * QSIZE,
                              kc * 128:(kc + 1) * 128])
            xq_tiles[q] = t

        # ---- startup: w blocks 0-1, x quarter 0, first prefetches ---------
        quant(0, w_d, wdq_d)
        quant(128, w_d, wdq_d)
        for t in range(XT_Q):
            quant(t * 128, x_d, xdq_d)
        emit_wT_reads(0)
        emit_xq_reads(0)

        # ---- main loop ----------------------------------------------------
        for q in range(QM):
            xqt = xq_tiles.pop(q)
            for pair in range(NPAIR):
                # producer interleave: quantize ahead of use
                if q == 0 and pair < NPAIR - 1:
                    quant((2 * pair + 2) * 128, w_d, wdq_d)
                    quant((2 * pair + 3) * 128, w_d, wdq_d)
                if q < QM - 1 and 2 <= pair < 2 + XT_Q:
                    quant((QSIZE * (q + 1)) + (pair - 2) * 128, x_d, xdq_d)
                # prefetch next pair's stationary tiles
                if pair < NPAIR - 1:
                    emit_wT_reads(pair + 1)
                elif q < QM - 1:
                    emit_xq_reads(q + 1)
                    emit_wT_reads(0)

                pmm = [ps.tile([128, 512], dt.float32, tag=f"ps{i}",
                               name=f"pmm{q}_{pair}_{i}") for i in range(4)]
                wts = pend_wT.pop(pair)
                for kc in range(KC):
                    wt = wts[kc]
                    for t in range(2):
                        for c in range(2):
                            nc.tensor.matmul(
                                pmm[2 * t + c][:, :],
                                wt[:, t * 128:(t + 1) * 128],
                                xqt[:, kc * QSIZE + c * 512:
                                    kc * QSIZE + (c + 1) * 512],
                                start=(kc == 0), stop=(kc == KC - 1),
                            )
                for t in range(2):
                    nt = 2 * pair + t
                    ob = osb.tile([128, QSIZE], dt.bfloat16, tag="osb",
                                  name=f"ob{q}_{nt}")
                    for c in range(2):
                        nc.scalar.activation(
                            ob[:, c * 512:(c + 1) * 512],
                            pmm[2 * t + c][:, :],
                            Act.Identity,
                            bias=bias_t[:, nt:nt + 1], scale=1.0,
                        )
                    nc.scalar.dma_start(
                        out=out_d[nt * 128:(nt + 1) * 128,
                                  q * QSIZE:(q + 1) * QSIZE],
                        in_=ob[:, :])
        assert not pend_wT and not xq_tiles

    if postprocess:
        _split_excess_waits(nc)
        # Raw Bass skips the ISA-byte encoding pass (Bacc.compile runs it);
        # without it custom-DVE/extended insts ship empty .instr -> walrus
        # "ISA wrong length".
        mybir.codegen_inst_isa_subclasses(nc)
    return nc


# ---------------------------------------------------------------------------
def _get_built():
    global _BUILT
    if _BUILT is None:
        _BUILT = build_nc()
    return _BUILT


def kernel(x, weight, bias):
    """Full-input entry point: x [2,4096,3072] f32, weight [12288,3072] f32,
    bias [12288] bf16 -> out [2,4096,12288] bf16."""
    from concourse.bass_utils import run_bass_kernel_spmd

    nc = _get_built()
    x2 = np.ascontiguousarray(np.asarray(x, dtype=f32).reshape(M, K))
    w = np.ascontiguousarray(np.asarray(weight, dtype=f32))
    b = np.asarray(bias)
    if b.dtype != bf16:
        if b.dtype.itemsize == 2 and b.dtype.kind in "Vu":
            b = b.view(bf16)
        else:
            b = b.astype(bf16)

    in_maps = []
    for c in range(NUM_CORES):
        mi, nj = divmod(c, GRID_N)
        in_maps.append({
            "x": x2[mi * M_CORE:(mi + 1) * M_CORE],
            "w": w[nj * N_CORE:(nj + 1) * N_CORE],
            "bias": b[nj * N_CORE:(nj + 1) * N_CORE],
        })

    res = run_bass_kernel_spmd(nc, in_maps, list(range(NUM_CORES)))
    out = np.empty((M, N), dtype=bf16)
    for c in range(NUM_CORES):
        mi, nj = divmod(c, GRID_N)
        # device output is [N_CORE, M_CORE] (transposed)
        oT = np.asarray(res.results[c]["out"])
        out[mi * M_CORE:(mi + 1) * M_CORE, nj * N_CORE:(nj + 1) * N_CORE] = (
            oT.astype(bf16, copy=False).T
        )
    return out.reshape(B, T, N)


# revision 22
# speedup vs baseline: 1.3307x; 1.3307x over previous
"""NVFP4 block-quantized linear layer (x @ w.T + bias) on 8 Trainium2 cores.

Reference semantics (reference.py): both activations and weights are
block-quantized along K (blocks of 16) to fp4-e2m1 with e4m3 scales
(scale = absmax/6, RNE), dequantized, then matmul with fp32 accumulation,
cast to bf16, plus bf16 bias.

v2 design (per core, 2-way M x 4-way N grid, out stored transposed [n, m]):
  - quantize x rows / w rows on VectorE+GpSimd: blockwise absmax reduce on
    GPSIMD, e4m3 RNE scale via exponent-mask + per-element magic-add
    (bitwise-identical to the reference chain, no reciprocal), fp4 round via
    two custom DVE ops, dequant to bf16 (exactly representable).
  - xdq/wdq staged to DRAM bf16, read back TRANSPOSED via the DMA xbar
    (dma_start_transpose) -- the PE does *only* matmuls (no transposes).
  - stationary = wdqT tile [k,128n], moving = xdqT [k, 512m] chunks; PSUM
    [n=128, m=512] fp32 accumulated over all 24 k-chunks; 8 banks = 2
    nt-pairs in flight (double-buffered).
  - evac: ScalarE activation(Identity) fuses fp32->bf16 cast + per-partition
    bias add (bias is per-n = per-partition in this orientation).
  - out written transposed [N_CORE, M_CORE]; the host reassembles.
"""

import os
import numpy as np
import ml_dtypes

f32 = np.float32
bf16 = ml_dtypes.bfloat16

# ---------------------------------------------------------------------------
# problem geometry (hardcoded; harness calls kernel() with these full shapes)
B, T, K = 2, 4096, 3072
N = 12288
M = B * T                      # 8192
GRID_M, GRID_N = 2, 4          # 8 cores
M_CORE = M // GRID_M           # 4096
N_CORE = N // GRID_N           # 3072
NUM_CORES = GRID_M * GRID_N

KC = K // 128                  # 24 k-chunks
KB = K // 16                   # 192 scale blocks per row
NT = N_CORE // 128             # 24 n-tiles
NPAIR = NT // 2                # 12 nt-pairs
QM = 4                         # m quarters
QSIZE = M_CORE // QM           # 1024
XT_Q = QSIZE // 128            # 8 x row-tiles per quarter

CH1 = float(1.5 * 2**22)
RCP6 = float(f32(1.0) / f32(6.0))
GPSIMD_REDUCE = False          # Pool engine rejects TensorTensor at codegen

_BUILT = None


# ---------------------------------------------------------------------------
def _register_custom_ops():
    """Register the two fp4-rounding custom DVE ops (idempotent)."""
    import concourse.dve_ops as dve_ops
    from concourse.dve_ops import DveOp, OPS, _SUB_OPCODE_FOR_NAME, _CUSTOM_DVE_ROW_BASE
    from concourse.dve_spec import (
        Spec, Src0, Src1, C0, C1, Zero, One, AluOp, Bin,
        maxx, minn, select, lower, _has_src1,
    )
    from concourse.dve_uop import DveOpSpec

    def _norm2(in0, in1):
        in0 = np.asarray(in0)
        in1 = np.asarray(in1)
        if in1.size != in0.size:
            in1 = np.broadcast_to(in1, in0.shape)
        return in0, np.ascontiguousarray(in1).reshape(in0.shape)

    def _ref_fp4_pre(in0, in1, s0, s1, imm2=None):
        in0, in1 = _norm2(in0, in1)
        m = (in0.astype(f32) * in1.astype(f32)).astype(f32)
        s2 = (m * m).astype(f32)
        ch = np.where(
            s2 < f32(4.0), f32(CH1),
            ((f32(1.0) + (s2 >= f32(16.0)).astype(f32)) * f32(1.5 * 2**23)).astype(f32),
        ).astype(f32)
        return (m + ch).astype(f32)

    def _ref_fp4_fin(in0, in1, s0, s1, imm2=None):
        in0, in1 = _norm2(in0, in1)
        qpre = np.ascontiguousarray(in0.astype(f32))
        pe = (qpre.view(np.uint32) & np.uint32(0x7F800000)).view(f32)
        d1 = (qpre - pe).astype(f32)
        q2 = ((d1 + d1).astype(f32) - pe).astype(f32)
        qc = np.maximum(np.minimum(q2, f32(12.0)), f32(-12.0))
        return (qc * in1.astype(f32)).astype(f32)

    def build_pre():
        SIXTEEN = C0 * C0
        Ch2x = C1 + C1
        m = Src0 * Src1
        s2 = m * m
        c2 = s2 >= SIXTEEN
        inner = (c2 + One) * Ch2x
        c1 = s2 < C0
        outer = select(c1, C1, inner)
        return Spec(body=m + outer, reference=_ref_fp4_pre)

    def build_fin():
        pe = Bin(AluOp.BITWISE_AND, Src0, C0)
        d1 = Src0 - pe
        q2 = (d1 + d1) - pe
        qc = maxx(minn(q2, C1), Zero - C1)
        return Spec(body=qc * Src1, reference=_ref_fp4_fin)

    def register(name, spec):
        if name in _SUB_OPCODE_FOR_NAME:
            for op in OPS:
                if op.name == name:
                    return op
            raise RuntimeError(name)
        row = _CUSTOM_DVE_ROW_BASE + len(OPS)
        assert row < 0x20
        shas = {}
        for ver in ("v3", "v4"):
            try:
                uops = lower(spec, ver=ver)
            except Exception:
                continue
            shas[ver] = DveOpSpec(
                name=name, opcode=row, uops=uops, rd1_en=_has_src1(spec)
            ).sha(ver)
        op = DveOp(name, spec, subdim=False, uops_sha=shas)
        OPS.append(op)
        _SUB_OPCODE_FOR_NAME[name] = row
        dve_ops.CUSTOM_DVE_SPECS[name] = spec
        return op

    return register("FP4_PRE_ANT", build_pre()), register("FP4_FIN_ANT", build_fin())


# ---------------------------------------------------------------------------
def _patch_tile_drain():
    """The TileContext tail drain attaches one sem-wait per live logical
    processor to a single SP Drain instruction; this walrus build caps sync
    waits per instruction at 2 ("Too many sync wait commands").  Split the
    overflow waits onto preceding single-wait SP nops (sound: all waits still
    complete before the post-drain all-engine barrier / sem reset)."""
    from concourse import tile as tile_mod
    import concourse.mybir as mybir
    from concourse.vector_clock import ScopedClock

    if getattr(tile_mod.TileContext, "_ant_drain_patched", False):
        return

    def _drain_and_barrier(self, tick_clock, wait_clock):
        nc = self.nc
        probe = nc.sync.nop()
        wait_clock.add_sem_waits(
            probe.ins, ScopedClock({None: tick_clock.global_clock})
        )
        si = probe.ins.sync_info
        waits = list(si.on_wait) if si is not None and si.on_wait else []
        if len(waits) > 1:
            probe.ins.sync_info = mybir.SyncInfo(
                on_wait=waits[:1],
                on_update=list(si.on_update) if si.on_update else [],
            )
            for w in waits[1:]:
                extra = nc.sync.nop()
                extra.ins.sync_info = mybir.SyncInfo(on_wait=[w], on_update=[])
        nc.sync.drain()

        nc.all_engine_barrier()
        assert self.sems is not None
        popped = nc._tile_sem_poison_stack.pop()
        assert popped is self._sem_poison
        nc.clear_and_free_semaphores(list(self.sems.allocated().values()))
        nc.all_engine_barrier()

    tile_mod.TileContext._drain_and_barrier = _drain_and_barrier
    tile_mod.TileContext._ant_drain_patched = True


def _split_excess_waits(nc, max_waits=1):
    """This walrus build rejects instructions carrying more than `max_waits`
    sem waits ("Too many sync wait commands").  Move overflow waits onto
    same-engine NoOp instructions inserted immediately before the offender —
    per-engine program order makes this semantically identical."""
    import concourse.mybir as mybir

    ctr = [0]
    for f in nc.m.functions:
        for blk in f.blocks:
            il = blk.instructions
            out = []
            changed = False
            for ins in il:
                si = ins.sync_info
                waits = list(si.on_wait) if si is not None and si.on_wait else []
                if len(waits) > max_waits:
                    changed = True
                    extra = waits[:-max_waits]
                    for i0 in range(0, len(extra), max_waits):
                        nop = mybir.InstNoOp(
                            name=f"I-waitsplit-{ctr[0]}", ins=[], outs=[])
                        ctr[0] += 1
                        nop.engine = ins.engine
                        nop.sync_info = mybir.SyncInfo(
                            on_wait=extra[i0:i0 + max_waits], on_update=[])
                        out.append(nop)
                    ins.sync_info = mybir.SyncInfo(
                        on_wait=waits[-max_waits:],
                        on_update=list(si.on_update) if si.on_update else [],
                    )
                out.append(ins)
            if changed:
                blk.instructions = out


def build_nc(m_core=M_CORE, k=K, n_core=N_CORE, num_cores=NUM_CORES,
             debug=False, postprocess=True, gpsimd_reduce=GPSIMD_REDUCE):
    """Build the per-core Bass program (SPMD: same program on every core)."""
    import concourse.bass as bass
    import concourse.mybir as mybir
    from concourse import tile
    from contextlib import ExitStack

    fp4_pre, fp4_fin = _register_custom_ops()
    _patch_tile_drain()

    nc = bass.Bass("TRN2", target_bir_lowering=False, debug=debug,
                   num_devices=num_cores)
    dt = mybir.dt
    Alu = mybir.AluOpType
    Act = mybir.ActivationFunctionType

    x_d = nc.dram_tensor("x", [m_core, k], dt.float32, kind="ExternalInput")
    w_d = nc.dram_tensor("w", [n_core, k], dt.float32, kind="ExternalInput")
    b_d = nc.dram_tensor("bias", [n_core], dt.bfloat16, kind="ExternalInput")
    out_d = nc.dram_tensor("out", [n_core, m_core], dt.bfloat16,
                           kind="ExternalOutput")

    with tile.TileContext(nc) as tc, ExitStack() as ctx:
        dram = ctx.enter_context(tc.tile_pool(name="dram", bufs=1, space="DRAM"))
        qin = ctx.enter_context(tc.tile_pool(name="qin", bufs=1))
        qout = ctx.enter_context(tc.tile_pool(name="qout", bufs=1))
        qpre = ctx.enter_context(tc.tile_pool(name="qpre", bufs=1))
        qtmp = ctx.enter_context(tc.tile_pool(name="qtmp", bufs=1))
        xq = ctx.enter_context(tc.tile_pool(name="xq", bufs=2))
        wT = ctx.enter_context(tc.tile_pool(name="wT", bufs=2))
        wtr = ctx.enter_context(tc.tile_pool(name="wtr", bufs=2))
        osb = ctx.enter_context(tc.tile_pool(name="osb", bufs=2))
        cst = ctx.enter_context(tc.tile_pool(name="cst", bufs=1))
        ps = ctx.enter_context(tc.tile_pool(name="ps", bufs=2, space="PSUM"))

        xdq_d = dram.tile([m_core, k], dt.bfloat16)
        wdq_d = dram.tile([n_core, k], dt.bfloat16)
        # wdqT staged pair-contiguous: [pair][kc][128 k][256 n]
        wdqT_d = dram.tile([NPAIR, KC, 128, 256], dt.bfloat16)

        # +inf per-partition scalar for FP4_FIN's exponent mask (an inf
        # *immediate* is not JSON-serializable through walrus)
        inf_t = cst.tile([128, 1], dt.float32, tag="inf")
        nc.vector.memset(inf_t[:, :], float("inf"))
        bias_t = cst.tile([128, NT], dt.bfloat16, tag="bias")
        nc.sync.dma_start(
            out=bias_t[:, :], in_=b_d[:].rearrange("(a p) -> p a", p=128))

        def quant(r0, src_d, dst_d, pair=2):
            """Quantize-dequantize rows [r0, r0+pair*128) of src_d into dst_d
            (pair rows-of-128 packed side-by-side on the free axis)."""
            kw = pair * k
            KBw = pair * KB
            xt = qin.tile([128, 2 * k], dt.float32, tag="qin", name="xt")
            src = src_d[r0:r0 + pair * 128, :].rearrange(
                "(t p) c -> p t c", t=pair)
            nc.scalar.dma_start(
                out=xt[:, :kw].rearrange("p (t c) -> p t c", t=pair), in_=src)
            x3 = xt[:, :kw].rearrange("p (b e) -> p b e", e=16)
            bm = qtmp.tile([128, 2 * KB], dt.float32, tag="bm", name="bm")
            nc.vector.tensor_reduce(
                bm[:, :KBw], x3, axis=mybir.AxisListType.X, op=Alu.max,
                apply_absolute_value=True,
            )
            # e4m3 RNE scale: sraw = max(bm/6, 2^-9); mt = 1.5*2^20 *
            # 2^clip(expo,-6..); s = (sraw + mt) - mt  (RNE to 2^(e-3) steps)
            sraw = qtmp.tile([128, 2 * KB], dt.float32, tag="sraw", name="sraw")
            nc.vector.tensor_scalar(
                sraw[:, :KBw], bm[:, :KBw], RCP6, float(2.0**-9),
                Alu.mult, Alu.max)
            pe = qtmp.tile([128, 2 * KB], dt.float32, tag="pe", name="pe")
            nc.vector.tensor_scalar(
                pe[:, :KBw].bitcast(dt.int32), sraw[:, :KBw].bitcast(dt.int32),
                0x7F800000, None, Alu.bitwise_and)
            mt = qtmp.tile([128, 2 * KB], dt.float32, tag="mt", name="mt")
            nc.vector.tensor_scalar(
                mt[:, :KBw], pe[:, :KBw], float(2.0**-6), float(1.5 * 2**20),
                Alu.max, Alu.mult)
            yt = qtmp.tile([128, 2 * KB], dt.float32, tag="yt", name="yt")
            nc.vector.tensor_tensor(
                yt[:, :KBw], sraw[:, :KBw], mt[:, :KBw], Alu.add)
            st = qtmp.tile([128, 2 * KB], dt.float32, tag="st", name="st")
            nc.vector.tensor_tensor(
                st[:, :KBw], yt[:, :KBw], mt[:, :KBw], Alu.subtract)
            sh = qtmp.tile([128, 2 * KB], dt.float32, tag="sh", name="sh")
            nc.vector.tensor_scalar_mul(sh[:, :KBw], st[:, :KBw], 0.5)
            rinv = qtmp.tile([128, 2 * KB], dt.float32, tag="rinv", name="rinv")
            nc.vector.reciprocal(rinv[:, :KBw], st[:, :KBw])

            qp = qpre.tile([128, 2 * k], dt.float32, tag="qpre", name="qp")
            qp3 = qp[:, :kw].rearrange("p (b e) -> p b e", e=16)
            nc.vector._custom_dve(
                fp4_pre, out=qp3, in0=x3,
                in1=rinv[:, :KBw].unsqueeze(2).broadcast_to([128, KBw, 16]),
                s0=4.0, s1=CH1,
            )
            dq = qout.tile([128, 2 * k], dt.bfloat16, tag="qout", name="dq")
            dq3 = dq[:, :kw].rearrange("p (b e) -> p b e", e=16)
            nc.vector._custom_dve(
                fp4_fin, out=dq3, in0=qp3,
                in1=sh[:, :KBw].unsqueeze(2).broadcast_to([128, KBw, 16]),
                s0=inf_t[:, 0:1], s1=12.0,
            )
            nc.scalar.dma_start(
                out=dst_d[r0:r0 + pair * 128, :].rearrange(
                    "(t p) c -> p t c", t=pair),
                in_=dq[:, :kw].rearrange("p (t c) -> p t c", t=pair))

        # ---- w transpose prep: wdq_d -> wdqT_d (pair-contiguous) ----------
        NGRP = 6                       # n-groups of 2 pairs (512 rows)
        GR = n_core // NGRP            # 512

        PAIRS_PER_GRP = GR // 256      # 2

        def emit_w_prep(g):
            """Transpose w rows [g*GR, (g+1)*GR) into wdqT_d, per kc."""
            p0 = g * PAIRS_PER_GRP
            for kc in range(KC):
                wt = wtr.tile([128, GR], dt.bfloat16, tag="wtr",
                              name=f"wtr{g}_{kc}")
                nc.sync.dma_start_transpose(
                    out=wt[:, :],
                    in_=wdq_d[g * GR:(g + 1) * GR, kc * 128:(kc + 1) * 128])
                nc.scalar.dma_start(
                    out=wdqT_d[p0:p0 + PAIRS_PER_GRP, kc, :, :].rearrange(
                        "r p j -> p r j"),
                    in_=wt[:, :].rearrange("p (r j) -> p r j", j=256))

        # ---- wdqT / xqT read helpers --------------------------------------
        pend_wT = {}

        def emit_wT_reads(pair):
            """Prefetch the stationary tiles for an nt-pair (one 1.5MB DMA)."""
            wt = wT.tile([128, KC * 256], dt.bfloat16, tag="wT",
                         name=f"wt{pair}")
            nc.sync.dma_start(
                out=wt[:, :].rearrange("p (c j) -> p c j", j=256),
                in_=wdqT_d[pair, :, :, :].rearrange("c p j -> p c j"))
            pend_wT[pair] = wt

        xq_tiles = {}

        def emit_xq_reads(q):
            """Read quarter q of xdq transposed: 24 slices [128k, 1024m]."""
            t = xq.tile([128, KC * QSIZE], dt.bfloat16, tag="xq",
                        name=f"xq{q}")
            for kc in range(KC):
                nc.sync.dma_start_transpose(
                    out=t[:, kc * QSIZE:(kc + 1) * QSIZE],
                    in_=xdq_d[q * QSIZE:(q + 1) * QSIZE,
                              kc * 128:(kc + 1) * 128])
            xq_tiles[q] = t

        # ---- startup: w group 0, x quarter 0, first prefetches ------------
        W_PER_GRP = (n_core // NGRP) // 128      # w row-blocks per prep group
        for wb in range(0, W_PER_GRP, 2):
            quant(wb * 128, w_d, wdq_d)
        emit_w_prep(0)
        next_wb = W_PER_GRP
        next_wg = 1
        for t in range(0, XT_Q, 2):
            quant(t * 128, x_d, xdq_d)
        emit_wT_reads(0)
        emit_xq_reads(0)

        # ---- main loop ----------------------------------------------------
        for q in range(QM):
            xqt = xq_tiles.pop(q)
            for pair in range(NPAIR):
                # producer interleave: quantize ahead of use
                if q == 0:
                    if next_wb < NT:
                        quant(next_wb * 128, w_d, wdq_d)
                        next_wb += 2
                    while (next_wg < NGRP
                           and next_wb >= (next_wg + 1) * W_PER_GRP):
                        emit_w_prep(next_wg)
                        next_wg += 1
                if q < QM - 1 and 2 <= pair < 2 + XT_Q // 2:
                    quant((QSIZE * (q + 1)) + (pair - 2) * 256, x_d, xdq_d)
                # prefetch next pair's stationary tiles
                if pair < NPAIR - 1:
                    emit_wT_reads(pair + 1)
                elif q < QM - 1:
                    emit_xq_reads(q + 1)
                    emit_wT_reads(0)

                pmm = [ps.tile([128, 512], dt.float32, tag=f"ps{i}",
                               name=f"pmm{q}_{pair}_{i}") for i in range(4)]
                wt = pend_wT.pop(pair)
                for kc in range(KC):
                    for t in range(2):
                        for c in range(2):
                            nc.tensor.matmul(
                                pmm[2 * t + c][:, :],
                                wt[:, kc * 256 + t * 128:
                                   kc * 256 + (t + 1) * 128],
                                xqt[:, kc * QSIZE + c * 512:
                                    kc * QSIZE + (c + 1) * 512],
                                start=(kc == 0), stop=(kc == KC - 1),
                            )
                for t in range(2):
                    nt = 2 * pair + t
                    ob = osb.tile([128, QSIZE], dt.bfloat16, tag="osb",
                                  name=f"ob{q}_{nt}")
                    for c in range(2):
                        nc.scalar.activation(
                            ob[:, c * 512:(c + 1) * 512],
                            pmm[2 * t + c][:, :],
                            Act.Identity,
                            bias=bias_t[:, nt:nt + 1], scale=1.0,
                        )
                    nc.scalar.dma_start(
                        out=out_d[nt * 128:(nt + 1) * 128,
                                  q * QSIZE:(q + 1) * QSIZE],
                        in_=ob[:, :])
        assert not pend_wT and not xq_tiles

    if postprocess:
        _split_excess_waits(nc)
        # Raw Bass skips the ISA-byte encoding pass (Bacc.compile runs it);
        # without it custom-DVE/extended insts ship empty .instr -> walrus
        # "ISA wrong length".
        mybir.codegen_inst_isa_subclasses(nc)
    return nc


# ---------------------------------------------------------------------------
def _get_built():
    global _BUILT
    if _BUILT is None:
        _BUILT = build_nc()
    return _BUILT


def kernel(x, weight, bias):
    """Full-input entry point: x [2,4096,3072] f32, weight [12288,3072] f32,
    bias [12288] bf16 -> out [2,4096,12288] bf16."""
    from concourse.bass_utils import run_bass_kernel_spmd

    nc = _get_built()
    x2 = np.ascontiguousarray(np.asarray(x, dtype=f32).reshape(M, K))
    w = np.ascontiguousarray(np.asarray(weight, dtype=f32))
    b = np.asarray(bias)
    if b.dtype != bf16:
        if b.dtype.itemsize == 2 and b.dtype.kind in "Vu":
            b = b.view(bf16)
        else:
            b = b.astype(bf16)

    in_maps = []
    for c in range(NUM_CORES):
        mi, nj = divmod(c, GRID_N)
        in_maps.append({
            "x": x2[mi * M_CORE:(mi + 1) * M_CORE],
            "w": w[nj * N_CORE:(nj + 1) * N_CORE],
            "bias": b[nj * N_CORE:(nj + 1) * N_CORE],
        })

    res = run_bass_kernel_spmd(nc, in_maps, list(range(NUM_CORES)))
    out = np.empty((M, N), dtype=bf16)
    for c in range(NUM_CORES):
        mi, nj = divmod(c, GRID_N)
        # device output is [N_CORE, M_CORE] (transposed)
        oT = np.asarray(res.results[c]["out"])
        out[mi * M_CORE:(mi + 1) * M_CORE, nj * N_CORE:(nj + 1) * N_CORE] = (
            oT.astype(bf16, copy=False).T
        )
    return out.reshape(B, T, N)


# revision 26
# speedup vs baseline: 1.3677x; 1.0278x over previous
"""NVFP4 block-quantized linear layer (x @ w.T + bias) on 8 Trainium2 cores.

Reference semantics (reference.py): both activations and weights are
block-quantized along K (blocks of 16) to fp4-e2m1 with e4m3 scales
(scale = absmax/6, RNE), dequantized, then matmul with fp32 accumulation,
cast to bf16, plus bf16 bias.

v2 design (per core, 2-way M x 4-way N grid, out stored transposed [n, m]):
  - quantize x rows / w rows on VectorE+GpSimd: blockwise absmax reduce on
    GPSIMD, e4m3 RNE scale via exponent-mask + per-element magic-add
    (bitwise-identical to the reference chain, no reciprocal), fp4 round via
    two custom DVE ops, dequant to bf16 (exactly representable).
  - xdq/wdq staged to DRAM bf16, read back TRANSPOSED via the DMA xbar
    (dma_start_transpose) -- the PE does *only* matmuls (no transposes).
  - stationary = wdqT tile [k,128n], moving = xdqT [k, 512m] chunks; PSUM
    [n=128, m=512] fp32 accumulated over all 24 k-chunks; 8 banks = 2
    nt-pairs in flight (double-buffered).
  - evac: ScalarE activation(Identity) fuses fp32->bf16 cast + per-partition
    bias add (bias is per-n = per-partition in this orientation).
  - out written transposed [N_CORE, M_CORE]; the host reassembles.
"""

import os
import numpy as np
import ml_dtypes

f32 = np.float32
bf16 = ml_dtypes.bfloat16

# ---------------------------------------------------------------------------
# problem geometry (hardcoded; harness calls kernel() with these full shapes)
B, T, K = 2, 4096, 3072
N = 12288
M = B * T                      # 8192
GRID_M, GRID_N = 2, 4          # 8 cores
M_CORE = M // GRID_M           # 4096
N_CORE = N // GRID_N           # 3072
NUM_CORES = GRID_M * GRID_N

KC = K // 128                  # 24 k-chunks
KB = K // 16                   # 192 scale blocks per row
NT = N_CORE // 128             # 24 n-tiles
NPAIR = NT // 2                # 12 nt-pairs
QM = 4                         # m quarters
QSIZE = M_CORE // QM           # 1024
XT_Q = QSIZE // 128            # 8 x row-tiles per quarter

CH1 = float(1.5 * 2**22)
RCP6 = float(f32(1.0) / f32(6.0))
GPSIMD_REDUCE = False          # Pool engine rejects TensorTensor at codegen

_BUILT = None


# ---------------------------------------------------------------------------
def _register_custom_ops():
    """Register the two fp4-rounding custom DVE ops (idempotent)."""
    import concourse.dve_ops as dve_ops
    from concourse.dve_ops import DveOp, OPS, _SUB_OPCODE_FOR_NAME, _CUSTOM_DVE_ROW_BASE
    from concourse.dve_spec import (
        Spec, Src0, Src1, C0, C1, Zero, One, AluOp, Bin,
        maxx, minn, select, lower, _has_src1,
    )
    from concourse.dve_uop import DveOpSpec

    def _norm2(in0, in1):
        in0 = np.asarray(in0)
        in1 = np.asarray(in1)
        if in1.size != in0.size:
            in1 = np.broadcast_to(in1, in0.shape)
        return in0, np.ascontiguousarray(in1).reshape(in0.shape)

    def _ref_fp4_pre(in0, in1, s0, s1, imm2=None):
        in0, in1 = _norm2(in0, in1)
        m = (in0.astype(f32) * in1.astype(f32)).astype(f32)
        s2 = (m * m).astype(f32)
        ch = np.where(
            s2 < f32(4.0), f32(CH1),
            ((f32(1.0) + (s2 >= f32(16.0)).astype(f32)) * f32(1.5 * 2**23)).astype(f32),
        ).astype(f32)
        return (m + ch).astype(f32)

    def _ref_fp4_fin(in0, in1, s0, s1, imm2=None):
        in0, in1 = _norm2(in0, in1)
        qpre = np.ascontiguousarray(in0.astype(f32))
        pe = (qpre.view(np.uint32) & np.uint32(0x7F800000)).view(f32)
        d1 = (qpre - pe).astype(f32)
        q2 = ((d1 + d1).astype(f32) - pe).astype(f32)
        qc = np.maximum(np.minimum(q2, f32(12.0)), f32(-12.0))
        return (qc * in1.astype(f32)).astype(f32)

    def build_pre():
        SIXTEEN = C0 * C0
        Ch2x = C1 + C1
        m = Src0 * Src1
        s2 = m * m
        c2 = s2 >= SIXTEEN
        inner = (c2 + One) * Ch2x
        c1 = s2 < C0
        outer = select(c1, C1, inner)
        return Spec(body=m + outer, reference=_ref_fp4_pre)

    def build_fin():
        pe = Bin(AluOp.BITWISE_AND, Src0, C0)
        d1 = Src0 - pe
        q2 = (d1 + d1) - pe
        qc = maxx(minn(q2, C1), Zero - C1)
        return Spec(body=qc * Src1, reference=_ref_fp4_fin)

    def register(name, spec):
        if name in _SUB_OPCODE_FOR_NAME:
            for op in OPS:
                if op.name == name:
                    return op
            raise RuntimeError(name)
        row = _CUSTOM_DVE_ROW_BASE + len(OPS)
        assert row < 0x20
        shas = {}
        for ver in ("v3", "v4"):
            try:
                uops = lower(spec, ver=ver)
            except Exception:
                continue
            shas[ver] = DveOpSpec(
                name=name, opcode=row, uops=uops, rd1_en=_has_src1(spec)
            ).sha(ver)
        op = DveOp(name, spec, subdim=False, uops_sha=shas)
        OPS.append(op)
        _SUB_OPCODE_FOR_NAME[name] = row
        dve_ops.CUSTOM_DVE_SPECS[name] = spec
        return op

    return register("FP4_PRE_ANT", build_pre()), register("FP4_FIN_ANT", build_fin())


# ---------------------------------------------------------------------------
def _patch_tile_drain():
    """The TileContext tail drain attaches one sem-wait per live logical
    processor to a single SP Drain instruction; this walrus build caps sync
    waits per instruction at 2 ("Too many sync wait commands").  Split the
    overflow waits onto preceding single-wait SP nops (sound: all waits still
    complete before the post-drain all-engine barrier / sem reset)."""
    from concourse import tile as tile_mod
    import concourse.mybir as mybir
    from concourse.vector_clock import ScopedClock

    if getattr(tile_mod.TileContext, "_ant_drain_patched", False):
        return

    def _drain_and_barrier(self, tick_clock, wait_clock):
        nc = self.nc
        probe = nc.sync.nop()
        wait_clock.add_sem_waits(
            probe.ins, ScopedClock({None: tick_clock.global_clock})
        )
        si = probe.ins.sync_info
        waits = list(si.on_wait) if si is not None and si.on_wait else []
        if len(waits) > 1:
            probe.ins.sync_info = mybir.SyncInfo(
                on_wait=waits[:1],
                on_update=list(si.on_update) if si.on_update else [],
            )
            for w in waits[1:]:
                extra = nc.sync.nop()
                extra.ins.sync_info = mybir.SyncInfo(on_wait=[w], on_update=[])
        nc.sync.drain()

        nc.all_engine_barrier()
        assert self.sems is not None
        popped = nc._tile_sem_poison_stack.pop()
        assert popped is self._sem_poison
        nc.clear_and_free_semaphores(list(self.sems.allocated().values()))
        nc.all_engine_barrier()

    tile_mod.TileContext._drain_and_barrier = _drain_and_barrier
    tile_mod.TileContext._ant_drain_patched = True


def _split_excess_waits(nc, max_waits=1):
    """This walrus build rejects instructions carrying more than `max_waits`
    sem waits ("Too many sync wait commands").  Move overflow waits onto
    same-engine NoOp instructions inserted immediately before the offender —
    per-engine program order makes this semantically identical."""
    import concourse.mybir as mybir

    ctr = [0]
    for f in nc.m.functions:
        for blk in f.blocks:
            il = blk.instructions
            out = []
            changed = False
            for ins in il:
                si = ins.sync_info
                waits = list(si.on_wait) if si is not None and si.on_wait else []
                if len(waits) > max_waits:
                    changed = True
                    extra = waits[:-max_waits]
                    for i0 in range(0, len(extra), max_waits):
                        nop = mybir.InstNoOp(
                            name=f"I-waitsplit-{ctr[0]}", ins=[], outs=[])
                        ctr[0] += 1
                        nop.engine = ins.engine
                        nop.sync_info = mybir.SyncInfo(
                            on_wait=extra[i0:i0 + max_waits], on_update=[])
                        out.append(nop)
                    ins.sync_info = mybir.SyncInfo(
                        on_wait=waits[-max_waits:],
                        on_update=list(si.on_update) if si.on_update else [],
                    )
                out.append(ins)
            if changed:
                blk.instructions = out


def build_nc(m_core=M_CORE, k=K, n_core=N_CORE, num_cores=NUM_CORES,
             debug=False, postprocess=True, gpsimd_reduce=GPSIMD_REDUCE):
    """Build the per-core Bass program (SPMD: same program on every core)."""
    import concourse.bass as bass
    import concourse.mybir as mybir
    from concourse import tile
    from contextlib import ExitStack

    fp4_pre, fp4_fin = _register_custom_ops()
    _patch_tile_drain()

    nc = bass.Bass("TRN2", target_bir_lowering=False, debug=debug,
                   num_devices=num_cores)
    dt = mybir.dt
    Alu = mybir.AluOpType
    Act = mybir.ActivationFunctionType

    x_d = nc.dram_tensor("x", [m_core, k], dt.float32, kind="ExternalInput")
    w_d = nc.dram_tensor("w", [n_core, k], dt.float32, kind="ExternalInput")
    b_d = nc.dram_tensor("bias", [n_core], dt.bfloat16, kind="ExternalInput")
    out_d = nc.dram_tensor("out", [n_core, m_core], dt.bfloat16,
                           kind="ExternalOutput")

    with tile.TileContext(nc) as tc, ExitStack() as ctx:
        dram = ctx.enter_context(tc.tile_pool(name="dram", bufs=1, space="DRAM"))
        qin = ctx.enter_context(tc.tile_pool(name="qin", bufs=2))
        qout = ctx.enter_context(tc.tile_pool(name="qout", bufs=1))
        qtmp = ctx.enter_context(tc.tile_pool(name="qtmp", bufs=1))
        xq = ctx.enter_context(tc.tile_pool(name="xq", bufs=2))
        wT = ctx.enter_context(tc.tile_pool(name="wT", bufs=2))
        wtr = ctx.enter_context(tc.tile_pool(name="wtr", bufs=2))
        osb = ctx.enter_context(tc.tile_pool(name="osb", bufs=2))
        cst = ctx.enter_context(tc.tile_pool(name="cst", bufs=1))
        ps = ctx.enter_context(tc.tile_pool(name="ps", bufs=2, space="PSUM"))

        xdq_d = dram.tile([m_core, k], dt.bfloat16)
        wdq_d = dram.tile([n_core, k], dt.bfloat16)
        # wdqT staged pair-contiguous: [pair][kc][128 k][256 n]
        wdqT_d = dram.tile([NPAIR, KC, 128, 256], dt.bfloat16)

        # +inf per-partition scalar for FP4_FIN's exponent mask (an inf
        # *immediate* is not JSON-serializable through walrus)
        inf_t = cst.tile([128, 1], dt.float32, tag="inf")
        nc.vector.memset(inf_t[:, :], float("inf"))
        bias_t = cst.tile([128, NT], dt.bfloat16, tag="bias")
        nc.sync.dma_start(
            out=bias_t[:, :], in_=b_d[:].rearrange("(a p) -> p a", p=128))

        def quant(r0, src_d, dst_d, pair=2):
            """Quantize-dequantize rows [r0, r0+pair*128) of src_d into dst_d
            (pair rows-of-128 packed side-by-side on the free axis)."""
            kw = pair * k
            KBw = pair * KB
            xt = qin.tile([128, 2 * k], dt.float32, tag="qin", name="xt")
            src = src_d[r0:r0 + pair * 128, :].rearrange(
                "(t p) c -> p t c", t=pair)
            nc.scalar.dma_start(
                out=xt[:, :kw].rearrange("p (t c) -> p t c", t=pair), in_=src)
            x3 = xt[:, :kw].rearrange("p (b e) -> p b e", e=16)
            bm = qtmp.tile([128, 2 * KB], dt.float32, tag="bm", name="bm")
            nc.vector.tensor_reduce(
                bm[:, :KBw], x3, axis=mybir.AxisListType.X, op=Alu.max,
                apply_absolute_value=True,
            )
            # e4m3 RNE scale: sraw = max(bm/6, 2^-9); mt = 1.5*2^20 *
            # 2^clip(expo,-6..); s = (sraw + mt) - mt  (RNE to 2^(e-3) steps)
            sraw = qtmp.tile([128, 2 * KB], dt.float32, tag="sraw", name="sraw")
            nc.vector.tensor_scalar(
                sraw[:, :KBw], bm[:, :KBw], RCP6, float(2.0**-9),
                Alu.mult, Alu.max)
            pe = qtmp.tile([128, 2 * KB], dt.float32, tag="pe", name="pe")
            nc.vector.tensor_scalar(
                pe[:, :KBw].bitcast(dt.int32), sraw[:, :KBw].bitcast(dt.int32),
                0x7F800000, None, Alu.bitwise_and)
            mt = qtmp.tile([128, 2 * KB], dt.float32, tag="mt", name="mt")
            nc.vector.tensor_scalar(
                mt[:, :KBw], pe[:, :KBw], float(2.0**-6), float(1.5 * 2**20),
                Alu.max, Alu.mult)
            yt = qtmp.tile([128, 2 * KB], dt.float32, tag="yt", name="yt")
            nc.vector.tensor_tensor(
                yt[:, :KBw], sraw[:, :KBw], mt[:, :KBw], Alu.add)
            st = qtmp.tile([128, 2 * KB], dt.float32, tag="st", name="st")
            nc.vector.tensor_tensor(
                st[:, :KBw], yt[:, :KBw], mt[:, :KBw], Alu.subtract)
            sh = qtmp.tile([128, 2 * KB], dt.float32, tag="sh", name="sh")
            nc.vector.tensor_scalar_mul(sh[:, :KBw], st[:, :KBw], 0.5)
            rinv = qtmp.tile([128, 2 * KB], dt.float32, tag="rinv", name="rinv")
            nc.vector.reciprocal(rinv[:, :KBw], st[:, :KBw])

            # fp4 round: PRE writes its packed-rounded output IN PLACE over
            # the input tile (exact elementwise overlap is safe on the
            # streaming DVE pipe), FIN dequantizes to bf16.
            nc.vector._custom_dve(
                fp4_pre, out=x3, in0=x3,
                in1=rinv[:, :KBw].unsqueeze(2).broadcast_to([128, KBw, 16]),
                s0=4.0, s1=CH1,
            )
            dq = qout.tile([128, 2 * k], dt.bfloat16, tag="qout", name="dq")
            dq3 = dq[:, :kw].rearrange("p (b e) -> p b e", e=16)
            nc.vector._custom_dve(
                fp4_fin, out=dq3, in0=x3,
                in1=sh[:, :KBw].unsqueeze(2).broadcast_to([128, KBw, 16]),
                s0=inf_t[:, 0:1], s1=12.0,
            )
            nc.scalar.dma_start(
                out=dst_d[r0:r0 + pair * 128, :].rearrange(
                    "(t p) c -> p t c", t=pair),
                in_=dq[:, :kw].rearrange("p (t c) -> p t c", t=pair))

        # ---- w transpose prep: wdq_d -> wdqT_d (pair-contiguous) ----------
        NGRP = 6                       # n-groups of 2 pairs (512 rows)
        GR = n_core // NGRP            # 512

        PAIRS_PER_GRP = GR // 256      # 2

        def emit_w_prep(g):
            """Transpose w rows [g*GR, (g+1)*GR) into wdqT_d, per kc."""
            p0 = g * PAIRS_PER_GRP
            for kc in range(KC):
                wt = wtr.tile([128, GR], dt.bfloat16, tag="wtr",
                              name=f"wtr{g}_{kc}")
                nc.sync.dma_start_transpose(
                    out=wt[:, :],
                    in_=wdq_d[g * GR:(g + 1) * GR, kc * 128:(kc + 1) * 128])
                nc.scalar.dma_start(
                    out=wdqT_d[p0:p0 + PAIRS_PER_GRP, kc, :, :].rearrange(
                        "r p j -> p r j"),
                    in_=wt[:, :].rearrange("p (r j) -> p r j", j=256))

        # ---- wdqT / xqT read helpers --------------------------------------
        pend_wT = {}

        def emit_wT_reads(pair):
            """Prefetch the stationary tiles for an nt-pair (one 1.5MB DMA)."""
            wt = wT.tile([128, KC * 256], dt.bfloat16, tag="wT",
                         name=f"wt{pair}")
            nc.sync.dma_start(
                out=wt[:, :].rearrange("p (c j) -> p c j", j=256),
                in_=wdqT_d[pair, :, :, :].rearrange("c p j -> p c j"))
            pend_wT[pair] = wt

        xq_tiles = {}

        def emit_xq_reads(q):
            """Read quarter q of xdq transposed: 24 slices [128k, 1024m]."""
            t = xq.tile([128, KC * QSIZE], dt.bfloat16, tag="xq",
                        name=f"xq{q}")
            for kc in range(KC):
                nc.sync.dma_start_transpose(
                    out=t[:, kc * QSIZE:(kc + 1) * QSIZE],
                    in_=xdq_d[q * QSIZE:(q + 1) * QSIZE,
                              kc * 128:(kc + 1) * 128])
            xq_tiles[q] = t

        # ---- startup: w group 0, x quarter 0, first prefetches ------------
        W_PER_GRP = (n_core // NGRP) // 128      # w row-blocks per prep group
        for wb in range(0, W_PER_GRP, 2):
            quant(wb * 128, w_d, wdq_d)
        emit_w_prep(0)
        next_wb = W_PER_GRP
        next_wg = 1
        for t in range(0, XT_Q, 2):
            quant(t * 128, x_d, xdq_d)
        emit_wT_reads(0)
        emit_xq_reads(0)

        # ---- main loop ----------------------------------------------------
        for q in range(QM):
            xqt = xq_tiles.pop(q)
            for pair in range(NPAIR):
                # producer interleave: front-load ALL w quant (each call
                # unlocks 2 pairs x every quarter of PE work), THEN x
                if q == 0:
                    for _ in range(2):
                        if next_wb < NT:
                            quant(next_wb * 128, w_d, wdq_d)
                            next_wb += 2
                    while (next_wg < NGRP
                           and next_wb >= (next_wg + 1) * W_PER_GRP):
                        emit_w_prep(next_wg)
                        next_wg += 1
                    if next_wb >= NT and 5 <= pair < 5 + XT_Q // 2:
                        quant(QSIZE + (pair - 5) * 256, x_d, xdq_d)
                elif q < QM - 1 and 2 <= pair < 2 + XT_Q // 2:
                    quant((QSIZE * (q + 1)) + (pair - 2) * 256, x_d, xdq_d)
                # prefetch next pair's stationary tiles
                if pair < NPAIR - 1:
                    emit_wT_reads(pair + 1)
                elif q < QM - 1:
                    emit_xq_reads(q + 1)
                    emit_wT_reads(0)

                pmm = [ps.tile([128, 512], dt.float32, tag=f"ps{i}",
                               name=f"pmm{q}_{pair}_{i}") for i in range(4)]
                wt = pend_wT.pop(pair)
                for kc in range(KC):
                    for t in range(2):
                        for c in range(2):
                            nc.tensor.matmul(
                                pmm[2 * t + c][:, :],
                                wt[:, kc * 256 + t * 128:
                                   kc * 256 + (t + 1) * 128],
                                xqt[:, kc * QSIZE + c * 512:
                                    kc * QSIZE + (c + 1) * 512],
                                start=(kc == 0), stop=(kc == KC - 1),
                            )
                for t in range(2):
                    nt = 2 * pair + t
                    ob = osb.tile([128, QSIZE], dt.bfloat16, tag="osb",
                                  name=f"ob{q}_{nt}")
                    for c in range(2):
                        nc.scalar.activation(
                            ob[:, c * 512:(c + 1) * 512],
                            pmm[2 * t + c][:, :],
                            Act.Identity,
                            bias=bias_t[:, nt:nt + 1], scale=1.0,
                        )
                    nc.scalar.dma_start(
                        out=out_d[nt * 128:(nt + 1) * 128,
                                  q * QSIZE:(q + 1) * QSIZE],
                        in_=ob[:, :])
        assert not pend_wT and not xq_tiles

    if postprocess:
        _split_excess_waits(nc)
        # Raw Bass skips the ISA-byte encoding pass (Bacc.compile runs it);
        # without it custom-DVE/extended insts ship empty .instr -> walrus
        # "ISA wrong length".
        mybir.codegen_inst_isa_subclasses(nc)
    return nc


# ---------------------------------------------------------------------------
def _get_built():
    global _BUILT
    if _BUILT is None:
        _BUILT = build_nc()
    return _BUILT


def kernel(x, weight, bias):
    """Full-input entry point: x [2,4096,3072] f32, weight [12288,3072] f32,
    bias [12288] bf16 -> out [2,4096,12288] bf16."""
    from concourse.bass_utils import run_bass_kernel_spmd

    nc = _get_built()
    x2 = np.ascontiguousarray(np.asarray(x, dtype=f32).reshape(M, K))
    w = np.ascontiguousarray(np.asarray(weight, dtype=f32))
    b = np.asarray(bias)
    if b.dtype != bf16:
        if b.dtype.itemsize == 2 and b.dtype.kind in "Vu":
            b = b.view(bf16)
        else:
            b = b.astype(bf16)

    in_maps = []
    for c in range(NUM_CORES):
        mi, nj = divmod(c, GRID_N)
        in_maps.append({
            "x": x2[mi * M_CORE:(mi + 1) * M_CORE],
            "w": w[nj * N_CORE:(nj + 1) * N_CORE],
            "bias": b[nj * N_CORE:(nj + 1) * N_CORE],
        })

    res = run_bass_kernel_spmd(nc, in_maps, list(range(NUM_CORES)))
    out = np.empty((M, N), dtype=bf16)
    for c in range(NUM_CORES):
        mi, nj = divmod(c, GRID_N)
        # device output is [N_CORE, M_CORE] (transposed)
        oT = np.asarray(res.results[c]["out"])
        out[mi * M_CORE:(mi + 1) * M_CORE, nj * N_CORE:(nj + 1) * N_CORE] = (
            oT.astype(bf16, copy=False).T
        )
    return out.reshape(B, T, N)


# revision 27
# speedup vs baseline: 1.4201x; 1.0383x over previous
"""NVFP4 block-quantized linear layer (x @ w.T + bias) on 8 Trainium2 cores.

Reference semantics (reference.py): both activations and weights are
block-quantized along K (blocks of 16) to fp4-e2m1 with e4m3 scales
(scale = absmax/6, RNE), dequantized, then matmul with fp32 accumulation,
cast to bf16, plus bf16 bias.

v2 design (per core, 2-way M x 4-way N grid, out stored transposed [n, m]):
  - quantize x rows / w rows on VectorE+GpSimd: blockwise absmax reduce on
    GPSIMD, e4m3 RNE scale via exponent-mask + per-element magic-add
    (bitwise-identical to the reference chain, no reciprocal), fp4 round via
    two custom DVE ops, dequant to bf16 (exactly representable).
  - xdq/wdq staged to DRAM bf16, read back TRANSPOSED via the DMA xbar
    (dma_start_transpose) -- the PE does *only* matmuls (no transposes).
  - stationary = wdqT tile [k,128n], moving = xdqT [k, 512m] chunks; PSUM
    [n=128, m=512] fp32 accumulated over all 24 k-chunks; 8 banks = 2
    nt-pairs in flight (double-buffered).
  - evac: ScalarE activation(Identity) fuses fp32->bf16 cast + per-partition
    bias add (bias is per-n = per-partition in this orientation).
  - out written transposed [N_CORE, M_CORE]; the host reassembles.
"""

import os
import numpy as np
import ml_dtypes

f32 = np.float32
bf16 = ml_dtypes.bfloat16

# ---------------------------------------------------------------------------
# problem geometry (hardcoded; harness calls kernel() with these full shapes)
B, T, K = 2, 4096, 3072
N = 12288
M = B * T                      # 8192
GRID_M, GRID_N = 2, 4          # 8 cores
M_CORE = M // GRID_M           # 4096
N_CORE = N // GRID_N           # 3072
NUM_CORES = GRID_M * GRID_N

KC = K // 128                  # 24 k-chunks
KB = K // 16                   # 192 scale blocks per row
NT = N_CORE // 128             # 24 n-tiles
NPAIR = NT // 2                # 12 nt-pairs
QM = 4                         # m quarters
QSIZE = M_CORE // QM           # 1024
XT_Q = QSIZE // 128            # 8 x row-tiles per quarter

CH1 = float(1.5 * 2**22)
RCP6 = float(f32(1.0) / f32(6.0))
GPSIMD_REDUCE = False          # Pool engine rejects TensorTensor at codegen

_BUILT = None


# ---------------------------------------------------------------------------
def _register_custom_ops():
    """Register the two fp4-rounding custom DVE ops (idempotent)."""
    import concourse.dve_ops as dve_ops
    from concourse.dve_ops import DveOp, OPS, _SUB_OPCODE_FOR_NAME, _CUSTOM_DVE_ROW_BASE
    from concourse.dve_spec import (
        Spec, Src0, Src1, C0, C1, Zero, One, AluOp, Bin,
        maxx, minn, select, lower, _has_src1,
    )
    from concourse.dve_uop import DveOpSpec

    def _norm2(in0, in1):
        in0 = np.asarray(in0)
        in1 = np.asarray(in1)
        if in1.size != in0.size:
            in1 = np.broadcast_to(in1, in0.shape)
        return in0, np.ascontiguousarray(in1).reshape(in0.shape)

    def _ref_fp4_pre(in0, in1, s0, s1, imm2=None):
        in0, in1 = _norm2(in0, in1)
        m = (in0.astype(f32) * in1.astype(f32)).astype(f32)
        s2 = (m * m).astype(f32)
        ch = np.where(
            s2 < f32(4.0), f32(CH1),
            ((f32(1.0) + (s2 >= f32(16.0)).astype(f32)) * f32(1.5 * 2**23)).astype(f32),
        ).astype(f32)
        return (m + ch).astype(f32)

    def _ref_fp4_fin(in0, in1, s0, s1, imm2=None):
        in0, in1 = _norm2(in0, in1)
        qpre = np.ascontiguousarray(in0.astype(f32))
        pe = (qpre.view(np.uint32) & np.uint32(0x7F800000)).view(f32)
        d1 = (qpre - pe).astype(f32)
        q2 = ((d1 + d1).astype(f32) - pe).astype(f32)
        qc = np.maximum(np.minimum(q2, f32(12.0)), f32(-12.0))
        return (qc * in1.astype(f32)).astype(f32)

    def build_pre():
        SIXTEEN = C0 * C0
        Ch2x = C1 + C1
        m = Src0 * Src1
        s2 = m * m
        c2 = s2 >= SIXTEEN
        inner = (c2 + One) * Ch2x
        c1 = s2 < C0
        outer = select(c1, C1, inner)
        return Spec(body=m + outer, reference=_ref_fp4_pre)

    def build_fin():
        pe = Bin(AluOp.BITWISE_AND, Src0, C0)
        d1 = Src0 - pe
        q2 = (d1 + d1) - pe
        qc = maxx(minn(q2, C1), Zero - C1)
        return Spec(body=qc * Src1, reference=_ref_fp4_fin)

    def register(name, spec):
        if name in _SUB_OPCODE_FOR_NAME:
            for op in OPS:
                if op.name == name:
                    return op
            raise RuntimeError(name)
        row = _CUSTOM_DVE_ROW_BASE + len(OPS)
        assert row < 0x20
        shas = {}
        for ver in ("v3", "v4"):
            try:
                uops = lower(spec, ver=ver)
            except Exception:
                continue
            shas[ver] = DveOpSpec(
                name=name, opcode=row, uops=uops, rd1_en=_has_src1(spec)
            ).sha(ver)
        op = DveOp(name, spec, subdim=False, uops_sha=shas)
        OPS.append(op)
        _SUB_OPCODE_FOR_NAME[name] = row
        dve_ops.CUSTOM_DVE_SPECS[name] = spec
        return op

    return register("FP4_PRE_ANT", build_pre()), register("FP4_FIN_ANT", build_fin())


# ---------------------------------------------------------------------------
def _patch_tile_drain():
    """The TileContext tail drain attaches one sem-wait per live logical
    processor to a single SP Drain instruction; this walrus build caps sync
    waits per instruction at 2 ("Too many sync wait commands").  Split the
    overflow waits onto preceding single-wait SP nops (sound: all waits still
    complete before the post-drain all-engine barrier / sem reset)."""
    from concourse import tile as tile_mod
    import concourse.mybir as mybir
    from concourse.vector_clock import ScopedClock

    if getattr(tile_mod.TileContext, "_ant_drain_patched", False):
        return

    def _drain_and_barrier(self, tick_clock, wait_clock):
        nc = self.nc
        probe = nc.sync.nop()
        wait_clock.add_sem_waits(
            probe.ins, ScopedClock({None: tick_clock.global_clock})
        )
        si = probe.ins.sync_info
        waits = list(si.on_wait) if si is not None and si.on_wait else []
        if len(waits) > 1:
            probe.ins.sync_info = mybir.SyncInfo(
                on_wait=waits[:1],
                on_update=list(si.on_update) if si.on_update else [],
            )
            for w in waits[1:]:
                extra = nc.sync.nop()
                extra.ins.sync_info = mybir.SyncInfo(on_wait=[w], on_update=[])
        nc.sync.drain()

        nc.all_engine_barrier()
        assert self.sems is not None
        popped = nc._tile_sem_poison_stack.pop()
        assert popped is self._sem_poison
        nc.clear_and_free_semaphores(list(self.sems.allocated().values()))
        nc.all_engine_barrier()

    tile_mod.TileContext._drain_and_barrier = _drain_and_barrier
    tile_mod.TileContext._ant_drain_patched = True


def _split_excess_waits(nc, max_waits=1):
    """This walrus build rejects instructions carrying more than `max_waits`
    sem waits ("Too many sync wait commands").  Move overflow waits onto
    same-engine NoOp instructions inserted immediately before the offender —
    per-engine program order makes this semantically identical."""
    import concourse.mybir as mybir

    ctr = [0]
    for f in nc.m.functions:
        for blk in f.blocks:
            il = blk.instructions
            out = []
            changed = False
            for ins in il:
                si = ins.sync_info
                waits = list(si.on_wait) if si is not None and si.on_wait else []
                if len(waits) > max_waits:
                    changed = True
                    extra = waits[:-max_waits]
                    for i0 in range(0, len(extra), max_waits):
                        nop = mybir.InstNoOp(
                            name=f"I-waitsplit-{ctr[0]}", ins=[], outs=[])
                        ctr[0] += 1
                        nop.engine = ins.engine
                        nop.sync_info = mybir.SyncInfo(
                            on_wait=extra[i0:i0 + max_waits], on_update=[])
                        out.append(nop)
                    ins.sync_info = mybir.SyncInfo(
                        on_wait=waits[-max_waits:],
                        on_update=list(si.on_update) if si.on_update else [],
                    )
                out.append(ins)
            if changed:
                blk.instructions = out


def build_nc(m_core=M_CORE, k=K, n_core=N_CORE, num_cores=NUM_CORES,
             debug=False, postprocess=True, gpsimd_reduce=GPSIMD_REDUCE):
    """Build the per-core Bass program (SPMD: same program on every core)."""
    import concourse.bass as bass
    import concourse.mybir as mybir
    from concourse import tile
    from contextlib import ExitStack

    fp4_pre, fp4_fin = _register_custom_ops()
    _patch_tile_drain()

    nc = bass.Bass("TRN2", target_bir_lowering=False, debug=debug,
                   num_devices=num_cores)
    dt = mybir.dt
    Alu = mybir.AluOpType
    Act = mybir.ActivationFunctionType

    x_d = nc.dram_tensor("x", [m_core, k], dt.float32, kind="ExternalInput")
    w_d = nc.dram_tensor("w", [n_core, k], dt.float32, kind="ExternalInput")
    b_d = nc.dram_tensor("bias", [n_core], dt.bfloat16, kind="ExternalInput")
    out_d = nc.dram_tensor("out", [n_core, m_core], dt.bfloat16,
                           kind="ExternalOutput")

    with tile.TileContext(nc) as tc, ExitStack() as ctx:
        dram = ctx.enter_context(tc.tile_pool(name="dram", bufs=1, space="DRAM"))
        qin = ctx.enter_context(tc.tile_pool(name="qin", bufs=2))
        qout = ctx.enter_context(tc.tile_pool(name="qout", bufs=1))
        qtmp = ctx.enter_context(tc.tile_pool(name="qtmp", bufs=1))
        xq = ctx.enter_context(tc.tile_pool(name="xq", bufs=2))
        wT = ctx.enter_context(tc.tile_pool(name="wT", bufs=2))
        wtr = ctx.enter_context(tc.tile_pool(name="wtr", bufs=2))
        osb = ctx.enter_context(tc.tile_pool(name="osb", bufs=2))
        cst = ctx.enter_context(tc.tile_pool(name="cst", bufs=1))
        ps = ctx.enter_context(tc.tile_pool(name="ps", bufs=2, space="PSUM"))

        xdq_d = dram.tile([m_core, k], dt.bfloat16)
        wdq_d = dram.tile([n_core, k], dt.bfloat16)
        # wdqT staged pair-contiguous: [pair][kc][128 k][256 n]
        wdqT_d = dram.tile([NPAIR, KC, 128, 256], dt.bfloat16)

        # +inf per-partition scalar for FP4_FIN's exponent mask (an inf
        # *immediate* is not JSON-serializable through walrus)
        inf_t = cst.tile([128, 1], dt.float32, tag="inf")
        nc.vector.memset(inf_t[:, :], float("inf"))
        bias_t = cst.tile([128, NT], dt.bfloat16, tag="bias")
        nc.sync.dma_start(
            out=bias_t[:, :], in_=b_d[:].rearrange("(a p) -> p a", p=128))

        def quant(r0, src_d, dst_d, pair=2):
            """Quantize-dequantize rows [r0, r0+pair*128) of src_d into dst_d
            (pair rows-of-128 packed side-by-side on the free axis)."""
            kw = pair * k
            KBw = pair * KB
            xt = qin.tile([128, 2 * k], dt.float32, tag="qin", name="xt")
            src = src_d[r0:r0 + pair * 128, :].rearrange(
                "(t p) c -> p t c", t=pair)
            # SWDGE: input loads must not share a HWDGE FIFO with the
            # DVE-dependent stores (head-of-line blocking stalls the pipeline)
            nc.gpsimd.dma_start(
                out=xt[:, :kw].rearrange("p (t c) -> p t c", t=pair), in_=src)
            x3 = xt[:, :kw].rearrange("p (b e) -> p b e", e=16)
            bm = qtmp.tile([128, 2 * KB], dt.float32, tag="bm", name="bm")
            nc.vector.tensor_reduce(
                bm[:, :KBw], x3, axis=mybir.AxisListType.X, op=Alu.max,
                apply_absolute_value=True,
            )
            # e4m3 RNE scale: sraw = max(bm/6, 2^-9); mt = 1.5*2^20 *
            # 2^clip(expo,-6..); s = (sraw + mt) - mt  (RNE to 2^(e-3) steps)
            sraw = qtmp.tile([128, 2 * KB], dt.float32, tag="sraw", name="sraw")
            nc.vector.tensor_scalar(
                sraw[:, :KBw], bm[:, :KBw], RCP6, float(2.0**-9),
                Alu.mult, Alu.max)
            pe = qtmp.tile([128, 2 * KB], dt.float32, tag="pe", name="pe")
            nc.vector.tensor_scalar(
                pe[:, :KBw].bitcast(dt.int32), sraw[:, :KBw].bitcast(dt.int32),
                0x7F800000, None, Alu.bitwise_and)
            mt = qtmp.tile([128, 2 * KB], dt.float32, tag="mt", name="mt")
            nc.vector.tensor_scalar(
                mt[:, :KBw], pe[:, :KBw], float(2.0**-6), float(1.5 * 2**20),
                Alu.max, Alu.mult)
            yt = qtmp.tile([128, 2 * KB], dt.float32, tag="yt", name="yt")
            nc.vector.tensor_tensor(
                yt[:, :KBw], sraw[:, :KBw], mt[:, :KBw], Alu.add)
            st = qtmp.tile([128, 2 * KB], dt.float32, tag="st", name="st")
            nc.vector.tensor_tensor(
                st[:, :KBw], yt[:, :KBw], mt[:, :KBw], Alu.subtract)
            sh = qtmp.tile([128, 2 * KB], dt.float32, tag="sh", name="sh")
            nc.vector.tensor_scalar_mul(sh[:, :KBw], st[:, :KBw], 0.5)
            rinv = qtmp.tile([128, 2 * KB], dt.float32, tag="rinv", name="rinv")
            nc.vector.reciprocal(rinv[:, :KBw], st[:, :KBw])

            # fp4 round: PRE writes its packed-rounded output IN PLACE over
            # the input tile (exact elementwise overlap is safe on the
            # streaming DVE pipe), FIN dequantizes to bf16.
            nc.vector._custom_dve(
                fp4_pre, out=x3, in0=x3,
                in1=rinv[:, :KBw].unsqueeze(2).broadcast_to([128, KBw, 16]),
                s0=4.0, s1=CH1,
            )
            dq = qout.tile([128, 2 * k], dt.bfloat16, tag="qout", name="dq")
            dq3 = dq[:, :kw].rearrange("p (b e) -> p b e", e=16)
            nc.vector._custom_dve(
                fp4_fin, out=dq3, in0=x3,
                in1=sh[:, :KBw].unsqueeze(2).broadcast_to([128, KBw, 16]),
                s0=inf_t[:, 0:1], s1=12.0,
            )
            nc.scalar.dma_start(
                out=dst_d[r0:r0 + pair * 128, :].rearrange(
                    "(t p) c -> p t c", t=pair),
                in_=dq[:, :kw].rearrange("p (t c) -> p t c", t=pair))

        # ---- w transpose prep: wdq_d -> wdqT_d (pair-contiguous) ----------
        NGRP = 6                       # n-groups of 2 pairs (512 rows)
        GR = n_core // NGRP            # 512

        PAIRS_PER_GRP = GR // 256      # 2

        def emit_w_prep(g):
            """Transpose w rows [g*GR, (g+1)*GR) into wdqT_d, per kc."""
            p0 = g * PAIRS_PER_GRP
            for kc in range(KC):
                wt = wtr.tile([128, GR], dt.bfloat16, tag="wtr",
                              name=f"wtr{g}_{kc}")
                nc.sync.dma_start_transpose(
                    out=wt[:, :],
                    in_=wdq_d[g * GR:(g + 1) * GR, kc * 128:(kc + 1) * 128])
                nc.scalar.dma_start(
                    out=wdqT_d[p0:p0 + PAIRS_PER_GRP, kc, :, :].rearrange(
                        "r p j -> p r j"),
                    in_=wt[:, :].rearrange("p (r j) -> p r j", j=256))

        # ---- wdqT / xqT read helpers --------------------------------------
        pend_wT = {}

        def emit_wT_reads(pair):
            """Prefetch the stationary tiles for an nt-pair (one 1.5MB DMA)."""
            wt = wT.tile([128, KC * 256], dt.bfloat16, tag="wT",
                         name=f"wt{pair}")
            nc.sync.dma_start(
                out=wt[:, :].rearrange("p (c j) -> p c j", j=256),
                in_=wdqT_d[pair, :, :, :].rearrange("c p j -> p c j"))
            pend_wT[pair] = wt

        xq_tiles = {}

        def emit_xq_reads(q):
            """Read quarter q of xdq transposed: 24 slices [128k, 1024m]."""
            t = xq.tile([128, KC * QSIZE], dt.bfloat16, tag="xq",
                        name=f"xq{q}")
            for kc in range(KC):
                nc.sync.dma_start_transpose(
                    out=t[:, kc * QSIZE:(kc + 1) * QSIZE],
                    in_=xdq_d[q * QSIZE:(q + 1) * QSIZE,
                              kc * 128:(kc + 1) * 128])
            xq_tiles[q] = t

        # ---- startup: w group 0, x quarter 0, first prefetches ------------
        W_PER_GRP = (n_core // NGRP) // 128      # w row-blocks per prep group
        for wb in range(0, W_PER_GRP, 2):
            quant(wb * 128, w_d, wdq_d)
        emit_w_prep(0)
        next_wb = W_PER_GRP
        next_wg = 1
        for t in range(0, XT_Q, 2):
            quant(t * 128, x_d, xdq_d)
        emit_wT_reads(0)
        emit_xq_reads(0)

        # ---- main loop ----------------------------------------------------
        for q in range(QM):
            xqt = xq_tiles.pop(q)
            for pair in range(NPAIR):
                # producer interleave: front-load ALL w quant (each call
                # unlocks 2 pairs x every quarter of PE work), THEN x
                if q == 0:
                    for _ in range(2):
                        if next_wb < NT:
                            quant(next_wb * 128, w_d, wdq_d)
                            next_wb += 2
                    while (next_wg < NGRP
                           and next_wb >= (next_wg + 1) * W_PER_GRP):
                        emit_w_prep(next_wg)
                        next_wg += 1
                    if next_wb >= NT and 5 <= pair < 5 + XT_Q // 2:
                        quant(QSIZE + (pair - 5) * 256, x_d, xdq_d)
                elif q < QM - 1 and 2 <= pair < 2 + XT_Q // 2:
                    quant((QSIZE * (q + 1)) + (pair - 2) * 256, x_d, xdq_d)
                # prefetch next pair's stationary tiles
                if pair < NPAIR - 1:
                    emit_wT_reads(pair + 1)
                elif q < QM - 1:
                    emit_xq_reads(q + 1)
                    emit_wT_reads(0)

                pmm = [ps.tile([128, 512], dt.float32, tag=f"ps{i}",
                               name=f"pmm{q}_{pair}_{i}") for i in range(4)]
                wt = pend_wT.pop(pair)
                for kc in range(KC):
                    for t in range(2):
                        for c in range(2):
                            nc.tensor.matmul(
                                pmm[2 * t + c][:, :],
                                wt[:, kc * 256 + t * 128:
                                   kc * 256 + (t + 1) * 128],
                                xqt[:, kc * QSIZE + c * 512:
                                    kc * QSIZE + (c + 1) * 512],
                                start=(kc == 0), stop=(kc == KC - 1),
                            )
                for t in range(2):
                    nt = 2 * pair + t
                    ob = osb.tile([128, QSIZE], dt.bfloat16, tag="osb",
                                  name=f"ob{q}_{nt}")
                    for c in range(2):
                        nc.scalar.activation(
                            ob[:, c * 512:(c + 1) * 512],
                            pmm[2 * t + c][:, :],
                            Act.Identity,
                            bias=bias_t[:, nt:nt + 1], scale=1.0,
                        )
                    nc.scalar.dma_start(
                        out=out_d[nt * 128:(nt + 1) * 128,
                                  q * QSIZE:(q + 1) * QSIZE],
                        in_=ob[:, :])
        assert not pend_wT and not xq_tiles

    if postprocess:
        _split_excess_waits(nc)
        # Raw Bass skips the ISA-byte encoding pass (Bacc.compile runs it);
        # without it custom-DVE/extended insts ship empty .instr -> walrus
        # "ISA wrong length".
        mybir.codegen_inst_isa_subclasses(nc)
    return nc


# ---------------------------------------------------------------------------
def _get_built():
    global _BUILT
    if _BUILT is None:
        _BUILT = build_nc()
    return _BUILT


def kernel(x, weight, bias):
    """Full-input entry point: x [2,4096,3072] f32, weight [12288,3072] f32,
    bias [12288] bf16 -> out [2,4096,12288] bf16."""
    from concourse.bass_utils import run_bass_kernel_spmd

    nc = _get_built()
    x2 = np.ascontiguousarray(np.asarray(x, dtype=f32).reshape(M, K))
    w = np.ascontiguousarray(np.asarray(weight, dtype=f32))
    b = np.asarray(bias)
    if b.dtype != bf16:
        if b.dtype.itemsize == 2 and b.dtype.kind in "Vu":
            b = b.view(bf16)
        else:
            b = b.astype(bf16)

    in_maps = []
    for c in range(NUM_CORES):
        mi, nj = divmod(c, GRID_N)
        in_maps.append({
            "x": x2[mi * M_CORE:(mi + 1) * M_CORE],
            "w": w[nj * N_CORE:(nj + 1) * N_CORE],
            "bias": b[nj * N_CORE:(nj + 1) * N_CORE],
        })

    res = run_bass_kernel_spmd(nc, in_maps, list(range(NUM_CORES)))
    out = np.empty((M, N), dtype=bf16)
    for c in range(NUM_CORES):
        mi, nj = divmod(c, GRID_N)
        # device output is [N_CORE, M_CORE] (transposed)
        oT = np.asarray(res.results[c]["out"])
        out[mi * M_CORE:(mi + 1) * M_CORE, nj * N_CORE:(nj + 1) * N_CORE] = (
            oT.astype(bf16, copy=False).T
        )
    return out.reshape(B, T, N)
